# Initial kernel scaffold
#
"""Trainium2 Bass kernel for a dense transformer block (nn_Block_120259084502).

Contract: kernel(**inputs) takes the FULL unsharded inputs (numpy, fp32) and
returns the FULL output [4, 2048, 1024] fp32. Internally shards across 8
NeuronCores: core c handles batch c//2, query-token half c%2. Each core
receives its batch's full 2048 tokens (rolled so its own 1024 query tokens
come first) and computes K/V for all of them locally, so no collectives are
needed (attention context = full batch; softmax is order-invariant so the
roll is harmless).

All heavy math runs on-device in bf16 matmuls with fp32 PSUM accumulation;
LayerNorm statistics and both residual adds stay fp32. LN gains and the
attention 1/sqrt(D) scale are folded into the weights on the host.
"""

import numpy as np
import ml_dtypes

import concourse.bacc as bacc
import concourse.tile as tile
from concourse import mybir
from concourse.bass_utils import run_bass_kernel_spmd
from concourse.masks import make_identity

bf16 = mybir.dt.bfloat16
f32 = mybir.dt.float32
AF = mybir.ActivationFunctionType
ALU = mybir.AluOpType

P = 128
B, T, E, H, D = 4, 2048, 1024, 16, 64
F = 4 * E                    # 4096 MLP hidden
TQ = T // 2                  # 1024 own query tokens per core
NE = E // P                  # 8 e-chunks
NPAIR = H // 2               # 8 head pairs
NST = T // P                 # 16 context-token tiles
NTS = TQ // P                # 8 own-token tiles
NF = F // P                  # 32 f-chunks
LN_EPS = 1e-5

_BUILD_CACHE = {}


def _emit_ln(nc, pools, xt, out_bf, eps_tile):
    """LayerNorm a token-major [128, E] fp32 tile -> bf16 (gain/bias folded
    into downstream weights by the host)."""
    stat, work = pools
    st = stat.tile([P, 2, nc.vector.BN_STATS_DIM], f32, name="bnst")
    xv = xt.rearrange("p (s g) -> p s g", s=2)
    nc.vector.bn_stats(out=st[:, 0, :], in_=xv[:, 0, :])
    nc.vector.bn_stats(out=st[:, 1, :], in_=xv[:, 1, :])
    mv = stat.tile([P, nc.vector.BN_AGGR_DIM], f32, name="bnmv")
    nc.vector.bn_aggr(out=mv, in_=st)
    rstd = stat.tile([P, 1], f32, name="bnrs")
    nc.scalar.activation(out=rstd, in_=mv[:, 1:2], func=AF.Sqrt, bias=eps_tile)
    nc.vector.reciprocal(out=rstd, in_=rstd)
    nc.vector.tensor_scalar(
        out=out_bf, in0=xt, scalar1=mv[:, 0:1], scalar2=rstd,
        op0=ALU.subtract, op1=ALU.mult,
    )


def _build(flags):
    """Build the SPMD Bass program. flags = (has_qb, has_pb, has_db)."""
    has_qb, has_pb, has_db = flags
    nc = bacc.Bacc("TRN2", target_bir_lowering=False, debug=False, num_devices=8)

    xkv_d = nc.dram_tensor("xkv", [T, E], f32, kind="ExternalInput")
    wq_d = nc.dram_tensor("wq", [NE, NPAIR, P, P], bf16, kind="ExternalInput")
    wk_d = nc.dram_tensor("wk", [NE, NPAIR, P, P], bf16, kind="ExternalInput")
    wv_d = nc.dram_tensor("wv", [NE, P, E], bf16, kind="ExternalInput")
    vrow_d = nc.dram_tensor("vrow", [1, H * (D + 1)], bf16, kind="ExternalInput")
    pw_d = nc.dram_tensor("pw", [NE, P, E], bf16, kind="ExternalInput")
    uw_d = nc.dram_tensor("uw", [NE, P, F], bf16, kind="ExternalInput")
    ub_d = nc.dram_tensor("ub", [P, NF], f32, kind="ExternalInput")
    dw_d = nc.dram_tensor("dw", [NF, P, E], bf16, kind="ExternalInput")
    if has_qb:
        qb_d = nc.dram_tensor("qb", [P, NPAIR], f32, kind="ExternalInput")
        kb_d = nc.dram_tensor("kb", [P, NPAIR], f32, kind="ExternalInput")
        vbrow_d = nc.dram_tensor("vbrow", [1, E], bf16, kind="ExternalInput")
    if has_pb:
        pbrow_d = nc.dram_tensor("pbrow", [1, E], f32, kind="ExternalInput")
    if has_db:
        dbrow_d = nc.dram_tensor("dbrow", [1, E], f32, kind="ExternalInput")
    out_d = nc.dram_tensor("out", [TQ, E], f32, kind="ExternalOutput")

    VW = D + 1  # per-head V width incl. the ones column (softmax denominator)

    with tile.TileContext(nc) as tc:
        with (
            tc.tile_pool(name="consts", bufs=1) as consts,
            tc.tile_pool(name="stat", bufs=4) as stat,
            tc.tile_pool(name="xq", bufs=1) as xqp,
            tc.tile_pool(name="hp", bufs=4) as hp,
            tc.tile_pool(name="hT", bufs=1) as hTp,
            tc.tile_pool(name="catT", bufs=1) as catp,
            tc.tile_pool(name="x2", bufs=1) as x2p,
            tc.tile_pool(name="h2T", bufs=1) as h2Tp,
        ):
            ident = consts.tile([P, P], bf16, name="ident")
            make_identity(nc, ident)
            eps_t = consts.tile([P, 1], f32, name="eps")
            nc.vector.memset(eps_t, LN_EPS)
            ub_sb = consts.tile([P, NF], f32, name="ubsb")
            nc.sync.dma_start(out=ub_sb, in_=ub_d[:, :])
            if has_qb:
                qb_sb = consts.tile([P, NPAIR], f32, name="qbsb")
                nc.sync.dma_start(out=qb_sb, in_=qb_d[:, :])
                kb_sb = consts.tile([P, NPAIR], f32, name="kbsb")
                nc.sync.dma_start(out=kb_sb, in_=kb_d[:, :])
                vb_bc = consts.tile([P, E], bf16, name="vbbc")
                nc.gpsimd.dma_start(
                    out=vb_bc, in_=vbrow_d.ap()[0:1, :].partition_broadcast(P)[:, 0, :]
                )
            if has_pb:
                pb_bc = consts.tile([P, E], f32, name="pbbc")
                nc.gpsimd.dma_start(
                    out=pb_bc, in_=pbrow_d.ap()[0:1, :].partition_broadcast(P)[:, 0, :]
                )
            if has_db:
                db_bc = consts.tile([P, E], f32, name="dbbc")
                nc.gpsimd.dma_start(
                    out=db_bc, in_=dbrow_d.ap()[0:1, :].partition_broadcast(P)[:, 0, :]
                )

            xq_tiles = [xqp.tile([P, E], f32, name=f"xq{i}") for i in range(NTS)]
            hT = [hTp.tile([P, T], bf16, name=f"hT{c}") for c in range(NE)]
            catT = [catp.tile([P, TQ], bf16, name=f"catT{p}") for p in range(NPAIR)]
            x2_tiles = [x2p.tile([P, E], f32, name=f"x2_{i}") for i in range(NTS)]
            h2T = [h2Tp.tile([P, TQ], bf16, name=f"h2T{c}") for c in range(NE)]

            # ---- Phase A+B: load x, LN1, transpose h -> hT (e-major) ----
            with (
                tc.tile_pool(name="xk", bufs=3) as xkp,
                tc.tile_pool(name="tps", bufs=2, space="PSUM") as tps,
            ):
                for i in range(NST):
                    if i < NTS:
                        xt = xq_tiles[i]
                    else:
                        xt = xkp.tile([P, E], f32, name="xk")
                    nc.sync.dma_start(out=xt, in_=xkv_d[i * P:(i + 1) * P, :])
                    ht = hp.tile([P, E], bf16, name="h")
                    _emit_ln(nc, (stat, None), xt, ht, eps_t)
                    for c in range(NE):
                        tp = tps.tile([P, P], bf16, name="tp")
                        nc.tensor.transpose(tp, ht[:, c * P:(c + 1) * P], ident)
                        nc.vector.tensor_copy(
                            out=hT[c][:, i * P:(i + 1) * P], in_=tp
                        )

            # ---- Phase C: V in natural [s, d] layout, augmented with ones ----
            with (
                tc.tile_pool(name="vaug", bufs=1) as vap,
                tc.tile_pool(name="wv", bufs=NE) as wvp,
                tc.tile_pool(name="vps", bufs=2, space="PSUM") as vps,
            ):
                va = [vap.tile([P, H * VW], bf16, name=f"va{s}") for s in range(NST)]
                wv_sb = []
                for c in range(NE):
                    w = wvp.tile([P, E], bf16, name=f"wv{c}")
                    nc.sync.dma_start(out=w, in_=wv_d[c])
                    wv_sb.append(w)
                for s in range(NST):
                    nc.gpsimd.dma_start(
                        out=va[s],
                        in_=vrow_d.ap()[0:1, :].partition_broadcast(P)[:, 0, :],
                    )
                    for j in range(2):
                        pv = vps.tile([P, 512], f32, name="pv")
                        for c in range(NE):
                            nc.tensor.matmul(
                                pv,
                                hT[c][:, s * P:(s + 1) * P],
                                wv_sb[c][:, j * 512:(j + 1) * 512],
                                start=(c == 0), stop=(c == NE - 1),
                            )
                        dst = va[s].rearrange("p (h c) -> p h c", c=VW)[
                            :, j * 8:(j + 1) * 8, 0:D
                        ]
                        src = pv.rearrange("p (h d) -> p h d", d=D)
                        if has_qb:
                            vb_view = vb_bc.rearrange("p (h d) -> p h d", d=D)[
                                :, j * 8:(j + 1) * 8, :
                            ]
                            nc.vector.tensor_add(out=dst, in0=src, in1=vb_view)
                        else:
                            nc.vector.tensor_copy(out=dst, in_=src)

                # ---- Phase D: per head-pair: Q^T, K^T, attention ----
                with (
                    tc.tile_pool(name="wqk", bufs=6) as wqkp,
                    tc.tile_pool(name="qt", bufs=2) as qtp,
                    tc.tile_pool(name="kt", bufs=2) as ktp,
                    tc.tile_pool(name="pt", bufs=4) as ptp,
                    tc.tile_pool(name="sm", bufs=4) as smp,
                    tc.tile_pool(name="qkps", bufs=2, space="PSUM") as qkps,
                    tc.tile_pool(name="scps", bufs=2, space="PSUM") as scps,
                    tc.tile_pool(name="atps", bufs=1, space="PSUM") as atps,
                ):
                    for p in range(NPAIR):
                        qt = qtp.tile([P, TQ], bf16, name="qt")
                        kt = ktp.tile([P, T], bf16, name="kt")
                        # Q^T for the pair: [128 (2 heads x 64d), 1024 tokens]
                        psq = [qkps.tile([P, 512], f32, name=f"qk{j}") for j in range(2)]
                        for c in range(NE):
                            wsl = wqkp.tile([P, P], bf16, name="wsl")
                            nc.sync.dma_start(out=wsl, in_=wq_d[c, p])
                            for j in range(2):
                                nc.tensor.matmul(
                                    psq[j], wsl, hT[c][:, j * 512:(j + 1) * 512],
                                    start=(c == 0), stop=(c == NE - 1),
                                )
                        for j in range(2):
                            dst = qt[:, j * 512:(j + 1) * 512]
                            if has_qb:
                                nc.vector.tensor_scalar(
                                    out=dst, in0=psq[j], scalar1=qb_sb[:, p:p + 1],
                                    op0=ALU.add,
                                )
                            else:
                                nc.vector.tensor_copy(out=dst, in_=psq[j])
                        # K^T for the pair: [128, 2048], two halves to cap PSUM
                        for sh in range(2):
                            psk = [qkps.tile([P, 512], f32, name=f"qk{j}") for j in range(2)]
                            for c in range(NE):
                                wsl = wqkp.tile([P, P], bf16, name="wsl")
                                nc.sync.dma_start(out=wsl, in_=wk_d[c, p])
                                for j in range(2):
                                    s0 = (sh * 2 + j) * 512
                                    nc.tensor.matmul(
                                        psk[j], wsl, hT[c][:, s0:s0 + 512],
                                        start=(c == 0), stop=(c == NE - 1),
                                    )
                            for j in range(2):
                                s0 = (sh * 2 + j) * 512
                                dst = kt[:, s0:s0 + 512]
                                if has_qb:
                                    nc.vector.tensor_scalar(
                                        out=dst, in0=psk[j], scalar1=kb_sb[:, p:p + 1],
                                        op0=ALU.add,
                                    )
                                else:
                                    nc.vector.tensor_copy(out=dst, in_=psk[j])

                        # attention for both heads of the pair
                        for th in range(2):
                            tcols = slice(th * 512, (th + 1) * 512)
                            at0 = atps.tile([D + 1, 512], f32, name="at0")
                            at1 = atps.tile([D + 1, 512], f32, name="at1")
                            for s in range(NST):
                                scols = slice(s * P, (s + 1) * P)
                                sc0 = scps.tile([P, 512], f32, name="sc0")
                                sc1 = scps.tile([P, 512], f32, name="sc1")
                                # S^T[s, t] = K^T.T @ Q^T ; heads packed on
                                # row-groups 0-63 / 64-127 (concurrent on PE)
                                nc.tensor.matmul(
                                    sc0, kt[0:D, scols], qt[0:D, tcols],
                                    start=True, stop=True,
                                )
                                nc.tensor.matmul(
                                    sc1, kt[D:2 * D, scols], qt[D:2 * D, tcols],
                                    start=True, stop=True,
                                )
                                pt0 = ptp.tile([P, 512], bf16, name="pt0")
                                pt1 = ptp.tile([P, 512], bf16, name="pt1")
                                nc.scalar.activation(out=pt0, in_=sc0, func=AF.Exp)
                                nc.scalar.activation(out=pt1, in_=sc1, func=AF.Exp)
                                # attn^T accumulation; row 64 = sum(exp)
                                nc.tensor.matmul(
                                    at0, va[s][:, (2 * p) * VW:(2 * p) * VW + VW], pt0,
                                    start=(s == 0), stop=(s == NST - 1),
                                )
                                nc.tensor.matmul(
                                    at1, va[s][:, (2 * p + 1) * VW:(2 * p + 1) * VW + VW], pt1,
                                    start=(s == 0), stop=(s == NST - 1),
                                )
                            se = smp.tile([2, 512], f32, name="se")
                            nc.vector.reciprocal(out=se[0:1, :], in_=at0[D:D + 1, :])
                            nc.vector.reciprocal(out=se[1:2, :], in_=at1[D:D + 1, :])
                            rb0 = smp.tile([D, 512], f32, name="rb0")
                            rb1 = smp.tile([D, 512], f32, name="rb1")
                            nc.gpsimd.dma_start(
                                out=rb0, in_=se[0:1, :].partition_broadcast(D)[:, 0, :]
                            )
                            nc.gpsimd.dma_start(
                                out=rb1, in_=se[1:2, :].partition_broadcast(D)[:, 0, :]
                            )
                            nc.vector.tensor_mul(
                                out=catT[p][0:D, tcols], in0=at0[0:D, :], in1=rb0
                            )
                            nc.vector.tensor_mul(
                                out=catT[p][D:2 * D, tcols], in0=at1[0:D, :], in1=rb1
                            )

            # ---- Phase E: proj + residual, LN2, transpose h2 -> h2T ----
            with (
                tc.tile_pool(name="pw", bufs=NE) as pwp,
                tc.tile_pool(name="uwsb", bufs=NE) as uwp,
                tc.tile_pool(name="h2", bufs=3) as h2p,
                tc.tile_pool(name="pps", bufs=2, space="PSUM") as pps,
                tc.tile_pool(name="t2ps", bufs=2, space="PSUM") as t2ps,
            ):
                pw_sb = []
                for c in range(NE):
                    w = pwp.tile([P, E], bf16, name=f"pw{c}")
                    nc.sync.dma_start(out=w, in_=pw_d[c])
                    pw_sb.append(w)
                # prefetch MLP up-weights while proj runs
                uw_sb = []
                for c in range(NE):
                    w = uwp.tile([P, F], bf16, name=f"uw{c}")
                    nc.sync.dma_start(out=w, in_=uw_d[c])
                    uw_sb.append(w)

                for ts in range(NTS):
                    trows = slice(ts * P, (ts + 1) * P)
                    psy = [pps.tile([P, 512], f32, name=f"py{j}") for j in range(2)]
                    for c in range(NE):
                        for j in range(2):
                            nc.tensor.matmul(
                                psy[j], catT[c][:, trows],
                                pw_sb[c][:, j * 512:(j + 1) * 512],
                                start=(c == 0), stop=(c == NE - 1),
                            )
                    x2 = x2_tiles[ts]
                    for j in range(2):
                        jc = slice(j * 512, (j + 1) * 512)
                        if has_pb:
                            nc.vector.tensor_add(out=x2[:, jc], in0=psy[j], in1=pb_bc[:, jc])
                            nc.vector.tensor_add(out=x2[:, jc], in0=x2[:, jc], in1=xq_tiles[ts][:, jc])
                        else:
                            nc.vector.tensor_add(out=x2[:, jc], in0=psy[j], in1=xq_tiles[ts][:, jc])
                    h2 = h2p.tile([P, E], bf16, name="h2")
                    _emit_ln(nc, (stat, None), x2, h2, eps_t)
                    for c in range(NE):
                        tp = t2ps.tile([P, P], bf16, name="t2")
                        nc.tensor.transpose(tp, h2[:, c * P:(c + 1) * P], ident)
                        nc.vector.tensor_copy(out=h2T[c][:, trows], in_=tp)

                # ---- Phase F: MLP up (relu fused) + down + residual ----
                with (
                    tc.tile_pool(name="hid", bufs=6) as hidp,
                    tc.tile_pool(name="dwp", bufs=4) as dwpp,
                    tc.tile_pool(name="outp", bufs=3) as outp,
                    tc.tile_pool(name="upps", bufs=3, space="PSUM") as upps,
                    tc.tile_pool(name="dnps", bufs=1, space="PSUM") as dnps,
                ):
                    TQQ = 256  # token quarter
                    for q in range(4):
                        qcols = slice(q * TQQ, (q + 1) * TQQ)
                        dn = [dnps.tile([P, E], f32, name=f"dn{j}") for j in range(2)]
                        for f in range(NF):
                            pu = upps.tile([P, TQQ], f32, name="pu")
                            for c in range(NE):
                                nc.tensor.matmul(
                                    pu, uw_sb[c][:, f * P:(f + 1) * P], h2T[c][:, qcols],
                                    start=(c == 0), stop=(c == NE - 1),
                                )
                            hid = hidp.tile([P, TQQ], bf16, name="hid")
                            nc.scalar.activation(
                                out=hid, in_=pu, func=AF.Relu, bias=ub_sb[:, f:f + 1]
                            )
                            dwt = dwpp.tile([P, E], bf16, name="dwt")
                            nc.sync.dma_start(out=dwt, in_=dw_d[f])
                            for t2 in range(2):
                                for j in range(2):
                                    nc.tensor.matmul(
                                        dn[t2][:, j * 512:(j + 1) * 512],
                                        hid[:, t2 * P:(t2 + 1) * P],
                                        dwt[:, j * 512:(j + 1) * 512],
                                        start=(f == 0), stop=(f == NF - 1),
                                    )
                        for t2 in range(2):
                            ti = q * 2 + t2
                            ot = outp.tile([P, E], f32, name="ot")
                            if has_db:
                                nc.vector.tensor_add(out=ot, in0=dn[t2], in1=db_bc)
                                nc.vector.tensor_add(out=ot, in0=ot, in1=x2_tiles[ti])
                            else:
                                nc.vector.tensor_add(out=ot, in0=dn[t2], in1=x2_tiles[ti])
                            nc.sync.dma_start(
                                out=out_d[ti * P:(ti + 1) * P, :], in_=ot
                            )

    nc.finalize()
    return nc


def _get_nc(flags):
    if flags not in _BUILD_CACHE:
        _BUILD_CACHE[flags] = _build(flags)
    return _BUILD_CACHE[flags]


def _prep(x, Wq, Wk, Wv, proj_w, proj_b, ln1_g, ln1_b, ln2_g, ln2_b,
          up_w, up_b, down_w, down_b):
    """Host-side shard + weight fold/cast/layout. Returns (flags, in_maps)."""
    bfl = ml_dtypes.bfloat16
    x = np.ascontiguousarray(np.asarray(x, dtype=np.float32))
    Wq = np.asarray(Wq, np.float32)
    Wk = np.asarray(Wk, np.float32)
    Wv = np.asarray(Wv, np.float32)
    g1 = np.asarray(ln1_g, np.float32)
    b1 = np.asarray(ln1_b, np.float32)
    g2 = np.asarray(ln2_g, np.float32)
    b2 = np.asarray(ln2_b, np.float32)
    proj_w = np.asarray(proj_w, np.float32)
    up_w = np.asarray(up_w, np.float32)
    down_w = np.asarray(down_w, np.float32)

    # [H, E, D] -> [E, H*D], fold attention scale into Q, LN1 gain into all
    wq_all = (Wq * (D ** -0.5)).transpose(1, 0, 2).reshape(E, E)
    wk_all = Wk.transpose(1, 0, 2).reshape(E, E)
    wv_all = Wv.transpose(1, 0, 2).reshape(E, E)
    qb_vec = b1 @ wq_all
    kb_vec = b1 @ wk_all
    vb_vec = b1 @ wv_all
    wq_f = g1[:, None] * wq_all
    wk_f = g1[:, None] * wk_all
    wv_f = g1[:, None] * wv_all

    def _pair_chunks(w):  # [E, E] -> [NE, NPAIR, P, P]
        return np.ascontiguousarray(
            w.reshape(NE, P, NPAIR, P).transpose(0, 2, 1, 3).astype(bfl)
        )

    wq_dev = _pair_chunks(wq_f)
    wk_dev = _pair_chunks(wk_f)
    wv_dev = np.ascontiguousarray(wv_f.reshape(NE, P, E).astype(bfl))

    vrow = np.zeros((1, H * (D + 1)), np.float32)
    vrow.reshape(H, D + 1)[:, D] = 1.0
    vrow = vrow.astype(bfl)

    pw_dev = np.ascontiguousarray(proj_w.reshape(NE, P, E).astype(bfl))
    uw_f = g2[:, None] * up_w
    uw_dev = np.ascontiguousarray(uw_f.reshape(NE, P, F).astype(bfl))
    ub_f = np.asarray(up_b, np.float32) + b2 @ up_w
    ub_dev = np.ascontiguousarray(ub_f.reshape(NF, P).T.astype(np.float32))
    dw_dev = np.ascontiguousarray(down_w.reshape(NF, P, E).astype(bfl))

    has_qb = bool(np.any(b1 != 0))
    has_pb = bool(np.any(np.asarray(proj_b) != 0))
    has_db = bool(np.any(np.asarray(down_b) != 0))
    flags = (has_qb, has_pb, has_db)

    shared = {
        "wq": wq_dev, "wk": wk_dev, "wv": wv_dev, "vrow": vrow,
        "pw": pw_dev, "uw": uw_dev, "ub": ub_dev, "dw": dw_dev,
    }
    if has_qb:
        shared["qb"] = np.ascontiguousarray(qb_vec.reshape(NPAIR, P).T.astype(np.float32))
        shared["kb"] = np.ascontiguousarray(kb_vec.reshape(NPAIR, P).T.astype(np.float32))
        shared["vbrow"] = vb_vec.reshape(1, E).astype(bfl)
    if has_pb:
        shared["pbrow"] = np.asarray(proj_b, np.float32).reshape(1, E)
    if has_db:
        shared["dbrow"] = np.asarray(down_b, np.float32).reshape(1, E)

    in_maps = []
    for c in range(8):
        b, half = c // 2, c % 2
        xb = x[b]
        if half == 1:
            xb = np.concatenate([xb[TQ:], xb[:TQ]], axis=0)
        in_maps.append({"xkv": np.ascontiguousarray(xb), **shared})
    return flags, in_maps


def kernel(**inputs) -> np.ndarray:
    flags, in_maps = _prep(**inputs)
    nc = _get_nc(flags)
    res = run_bass_kernel_spmd(nc, in_maps, core_ids=list(range(8)))
    out = np.empty((B, T, E), np.float32)
    for c in range(8):
        b, half = c // 2, c % 2
        out[b, half * TQ:(half + 1) * TQ, :] = res.results[c]["out"]
    return out


# revision 9
# speedup vs baseline: 76.9844x; 76.9844x over previous
"""Trainium2 Bass kernel for a dense transformer block (nn_Block_120259084502).

Contract: kernel(**inputs) takes the FULL unsharded inputs (numpy, fp32) and
returns the FULL output [4, 2048, 1024] fp32. Internally shards across 8
NeuronCores: core c handles batch c//2, query-token half c%2. Each core
receives its batch's full 2048 tokens (rolled so its own 1024 query tokens
come first) and computes K/V for all of them locally, so no collectives are
needed (attention context = full batch; softmax is order-invariant so the
roll is harmless).

All heavy math runs on-device in bf16 matmuls with fp32 PSUM accumulation;
LayerNorm statistics and both residual adds stay fp32. LN gains and the
attention 1/sqrt(D) scale are folded into the weights on the host.
"""

import numpy as np
import ml_dtypes

import concourse.bacc as bacc
import concourse.tile as tile
from concourse import mybir
from concourse.bass_utils import run_bass_kernel_spmd
from concourse.masks import make_identity

bf16 = mybir.dt.bfloat16
f32 = mybir.dt.float32
AF = mybir.ActivationFunctionType
ALU = mybir.AluOpType

P = 128
B, T, E, H, D = 4, 2048, 1024, 16, 64
F = 4 * E                    # 4096 MLP hidden
TQ = T // 2                  # 1024 own query tokens per core
NE = E // P                  # 8 e-chunks
NPAIR = H // 2               # 8 head pairs
NST = T // P                 # 16 context-token tiles
NTS = TQ // P                # 8 own-token tiles
NF = F // P                  # 32 f-chunks
VW = D + 1                   # per-head V width incl. ones column
LN_EPS = 1e-5

_BUILD_CACHE = {}


class _Ctx:
    """Shared build state passed between phase emitters."""
    pass


def _emit_ln(g, xt, out_bf):
    nc = g.nc
    st = g.stat.tile([P, 2, nc.vector.BN_STATS_DIM], f32, name="bnst")
    xv = xt.rearrange("p (s g) -> p s g", s=2)
    nc.vector.bn_stats(out=st[:, 0, :], in_=xv[:, 0, :])
    nc.vector.bn_stats(out=st[:, 1, :], in_=xv[:, 1, :])
    mv = g.stat.tile([P, nc.vector.BN_AGGR_DIM], f32, name="bnmv")
    nc.vector.bn_aggr(out=mv, in_=st)
    rstd = g.stat.tile([P, 1], f32, name="bnrs")
    nc.scalar.activation(out=rstd, in_=mv[:, 1:2], func=AF.Sqrt, bias=g.eps_t)
    nc.vector.reciprocal(out=rstd, in_=rstd)
    nc.vector.tensor_scalar(
        out=out_bf, in0=xt, scalar1=mv[:, 0:1], scalar2=rstd,
        op0=ALU.subtract, op1=ALU.mult,
    )


def _emit_consts(g):
    nc, consts = g.nc, g.consts
    g.ident = consts.tile([P, P], bf16, name="ident")
    make_identity(nc, g.ident)
    g.eps_t = consts.tile([P, 1], f32, name="eps")
    nc.vector.memset(g.eps_t, LN_EPS)
    g.ub_sb = consts.tile([P, NF], f32, name="ubsb")
    nc.sync.dma_start(out=g.ub_sb, in_=g.ub_d[:, :])
    if g.has_qb:
        g.qb_sb = consts.tile([P, NPAIR], f32, name="qbsb")
        nc.sync.dma_start(out=g.qb_sb, in_=g.qb_d[:, :])
        g.kb_sb = consts.tile([P, NPAIR], f32, name="kbsb")
        nc.sync.dma_start(out=g.kb_sb, in_=g.kb_d[:, :])
        g.vb_bc = consts.tile([P, E], bf16, name="vbbc")
        nc.gpsimd.dma_start(
            out=g.vb_bc, in_=g.vbrow_d.ap()[0:1, :].partition_broadcast(P)[:, 0, :]
        )
    if g.has_pb:
        g.pb_bc = consts.tile([P, E], f32, name="pbbc")
        nc.gpsimd.dma_start(
            out=g.pb_bc, in_=g.pbrow_d.ap()[0:1, :].partition_broadcast(P)[:, 0, :]
        )
    if g.has_db:
        g.db_bc = consts.tile([P, E], f32, name="dbbc")
        nc.gpsimd.dma_start(
            out=g.db_bc, in_=g.dbrow_d.ap()[0:1, :].partition_broadcast(P)[:, 0, :]
        )


def _emit_ln1_transpose(g, xkp, tps):
    """Load x, LN1, PE-transpose h into e-major hT."""
    nc = g.nc
    for i in range(NST):
        xt = xkp.tile([P, E], f32, name="xk")
        nc.sync.dma_start(out=xt, in_=g.xkv_d[i * P:(i + 1) * P, :])
        ht = g.hp.tile([P, E], bf16, name="h")
        _emit_ln(g, xt, ht)
        for c in range(NE):
            tp = tps.tile([P, P], bf16, name="tp")
            nc.tensor.transpose(tp, ht[:, c * P:(c + 1) * P], g.ident)
            nc.vector.tensor_copy(out=g.hT[c][:, i * P:(i + 1) * P], in_=tp)


def _emit_v(g, wvp, vps):
    """V in natural [s, d] layout for all heads, with ones column per head."""
    nc = g.nc
    wv_sb = []
    for c in range(NE):
        w = wvp.tile([P, E], bf16, name=f"wv{c}")
        nc.sync.dma_start(out=w, in_=g.wv_d[c])
        wv_sb.append(w)
    for s in range(NST):
        nc.gpsimd.dma_start(
            out=g.va[s],
            in_=g.vrow_d.ap()[0:1, :].partition_broadcast(P)[:, 0, :],
        )
        for j in range(2):
            pv = vps.tile([P, 512], f32, name="pv")
            for c in range(NE):
                nc.tensor.matmul(
                    pv, g.hT[c][:, s * P:(s + 1) * P],
                    wv_sb[c][:, j * 512:(j + 1) * 512],
                    start=(c == 0), stop=(c == NE - 1),
                )
            dst = g.va[s].rearrange("p (h c) -> p h c", c=VW)[
                :, j * 8:(j + 1) * 8, 0:D
            ]
            src = pv.rearrange("p (h d) -> p h d", d=D)
            if g.has_qb:
                vb_view = g.vb_bc.rearrange("p (h d) -> p h d", d=D)[
                    :, j * 8:(j + 1) * 8, :
                ]
                nc.vector.tensor_add(out=dst, in0=src, in1=vb_view)
            else:
                nc.vector.tensor_copy(out=dst, in_=src)


def _emit_qkt_pair(g, p, qt, kt, wqkp, qkps):
    """Q^T and K^T for head pair p: [128 (2 heads x 64d), tokens]."""
    nc = g.nc
    psq = [qkps.tile([P, 512], f32, name=f"qk{j}") for j in range(2)]
    for c in range(NE):
        wsl = wqkp.tile([P, P], bf16, name="wsl")
        nc.sync.dma_start(out=wsl, in_=g.wq_d[c, p])
        for j in range(2):
            nc.tensor.matmul(
                psq[j], wsl, g.hT[c][:, j * 512:(j + 1) * 512],
                start=(c == 0), stop=(c == NE - 1),
            )
    for j in range(2):
        dst = qt[:, j * 512:(j + 1) * 512]
        if g.has_qb:
            nc.vector.tensor_scalar(
                out=dst, in0=psq[j], scalar1=g.qb_sb[:, p:p + 1], op0=ALU.add
            )
        else:
            nc.vector.tensor_copy(out=dst, in_=psq[j])
    for sh in range(2):
        psk = [qkps.tile([P, 512], f32, name=f"qk{j}") for j in range(2)]
        for c in range(NE):
            wsl = wqkp.tile([P, P], bf16, name="wsl")
            nc.sync.dma_start(out=wsl, in_=g.wk_d[c, p])
            for j in range(2):
                s0 = (sh * 2 + j) * 512
                nc.tensor.matmul(
                    psk[j], wsl, g.hT[c][:, s0:s0 + 512],
                    start=(c == 0), stop=(c == NE - 1),
                )
        for j in range(2):
            s0 = (sh * 2 + j) * 512
            dst = kt[:, s0:s0 + 512]
            if g.has_qb:
                nc.vector.tensor_scalar(
                    out=dst, in0=psk[j], scalar1=g.kb_sb[:, p:p + 1], op0=ALU.add
                )
            else:
                nc.vector.tensor_copy(out=dst, in_=psk[j])


def _emit_attn_pair(g, p, qt, kt, ptp, smp, drp, scps, atps):
    """Scores (transposed), exp, attn^T + softmax denom, normalize -> catT."""
    nc = g.nc
    for th in range(2):
        tcols = slice(th * 512, (th + 1) * 512)
        at0 = atps.tile([D + 1, 512], f32, name="at0")
        at1 = atps.tile([D + 1, 512], f32, name="at1")
        for s in range(NST):
            scols = slice(s * P, (s + 1) * P)
            sc0 = scps.tile([P, 512], f32, name="sc0")
            sc1 = scps.tile([P, 512], f32, name="sc1")
            # S^T[s,t] = (K^T slice).T @ Q^T slice; the two heads live on
            # row-groups 0-63 / 64-127 so the matmuls pack concurrently.
            nc.tensor.matmul(sc0, kt[0:D, scols], qt[0:D, tcols],
                             start=True, stop=True)
            nc.tensor.matmul(sc1, kt[D:2 * D, scols], qt[D:2 * D, tcols],
                             start=True, stop=True)
            pt0 = ptp.tile([P, 512], bf16, name="pt0")
            pt1 = ptp.tile([P, 512], bf16, name="pt1")
            nc.scalar.activation(out=pt0, in_=sc0, func=AF.Exp)
            nc.scalar.activation(out=pt1, in_=sc1, func=AF.Exp)
            nc.tensor.matmul(
                at0, g.va[s][:, (2 * p) * VW:(2 * p) * VW + VW], pt0,
                start=(s == 0), stop=(s == NST - 1),
            )
            nc.tensor.matmul(
                at1, g.va[s][:, (2 * p + 1) * VW:(2 * p + 1) * VW + VW], pt1,
                start=(s == 0), stop=(s == NST - 1),
            )
        se0 = smp.tile([1, 512], f32, name="se0")
        se1 = smp.tile([1, 512], f32, name="se1")
        nc.vector.reciprocal(out=se0, in_=at0[D:D + 1, :])
        nc.vector.reciprocal(out=se1, in_=at1[D:D + 1, :])
        # partition-broadcast needs a DRAM source: bounce through scratch
        sed = drp.tile([2, 512], f32, name="sed")
        nc.gpsimd.dma_start(out=sed[0:1, :], in_=se0)
        nc.gpsimd.dma_start(out=sed[1:2, :], in_=se1)
        rb0 = smp.tile([D, 512], f32, name="rb0")
        rb1 = smp.tile([D, 512], f32, name="rb1")
        nc.gpsimd.dma_start(out=rb0, in_=sed[0:1, :].partition_broadcast(D)[:, 0, :])
        nc.gpsimd.dma_start(out=rb1, in_=sed[1:2, :].partition_broadcast(D)[:, 0, :])
        nc.vector.tensor_mul(out=g.catT[p][0:D, tcols], in0=at0[0:D, :], in1=rb0)
        nc.vector.tensor_mul(out=g.catT[p][D:2 * D, tcols], in0=at1[0:D, :], in1=rb1)


def _emit_proj_ln2(g, pwp, uwp, xq2p, h2p, pps, t2ps):
    nc = g.nc
    pw_sb = []
    for c in range(NE):
        w = pwp.tile([P, E], bf16, name=f"pw{c}")
        nc.sync.dma_start(out=w, in_=g.pw_d[c])
        pw_sb.append(w)
    g.uw_sb = []
    for c in range(NE):  # prefetch MLP up-weights while proj runs
        w = uwp.tile([P, F], bf16, name=f"uw{c}")
        nc.sync.dma_start(out=w, in_=g.uw_d[c])
        g.uw_sb.append(w)
    for ts in range(NTS):
        trows = slice(ts * P, (ts + 1) * P)
        xres = xq2p.tile([P, E], f32, name="xres")
        nc.sync.dma_start(out=xres, in_=g.xkv_d[ts * P:(ts + 1) * P, :])
        psy = [pps.tile([P, 512], f32, name=f"py{j}") for j in range(2)]
        for c in range(NE):
            for j in range(2):
                nc.tensor.matmul(
                    psy[j], g.catT[c][:, trows],
                    pw_sb[c][:, j * 512:(j + 1) * 512],
                    start=(c == 0), stop=(c == NE - 1),
                )
        x2 = g.x2_tiles[ts]
        for j in range(2):
            jc = slice(j * 512, (j + 1) * 512)
            if g.has_pb:
                nc.vector.tensor_add(out=x2[:, jc], in0=psy[j], in1=g.pb_bc[:, jc])
                nc.vector.tensor_add(out=x2[:, jc], in0=x2[:, jc],
                                     in1=xres[:, jc])
            else:
                nc.vector.tensor_add(out=x2[:, jc], in0=psy[j],
                                     in1=xres[:, jc])
        h2 = h2p.tile([P, E], bf16, name="h2")
        _emit_ln(g, x2, h2)
        for c in range(NE):
            tp = t2ps.tile([P, P], bf16, name="t2")
            nc.tensor.transpose(tp, h2[:, c * P:(c + 1) * P], g.ident)
            nc.vector.tensor_copy(out=g.h2T[c][:, trows], in_=tp)


def _emit_mlp(g, hidp, dwpp, outp, upps, dnps):
    nc = g.nc
    TQQ = 256  # token quarter
    for q in range(4):
        qcols = slice(q * TQQ, (q + 1) * TQQ)
        dn = [dnps.tile([P, E], f32, name=f"dn{j}") for j in range(2)]
        for f in range(NF):
            pu = upps.tile([P, TQQ], f32, name="pu")
            for c in range(NE):
                nc.tensor.matmul(
                    pu, g.uw_sb[c][:, f * P:(f + 1) * P], g.h2T[c][:, qcols],
                    start=(c == 0), stop=(c == NE - 1),
                )
            hid = hidp.tile([P, TQQ], bf16, name="hid")
            nc.scalar.activation(out=hid, in_=pu, func=AF.Relu,
                                 bias=g.ub_sb[:, f:f + 1])
            dwt = dwpp.tile([P, E], bf16, name="dwt")
            nc.sync.dma_start(out=dwt, in_=g.dw_d[f])
            for t2 in range(2):
                for j in range(2):
                    nc.tensor.matmul(
                        dn[t2][:, j * 512:(j + 1) * 512],
                        hid[:, t2 * P:(t2 + 1) * P],
                        dwt[:, j * 512:(j + 1) * 512],
                        start=(f == 0), stop=(f == NF - 1),
                    )
        for t2 in range(2):
            ti = q * 2 + t2
            ot = outp.tile([P, E], f32, name="ot")
            if g.has_db:
                nc.vector.tensor_add(out=ot, in0=dn[t2], in1=g.db_bc)
                nc.vector.tensor_add(out=ot, in0=ot, in1=g.x2_tiles[ti])
            else:
                nc.vector.tensor_add(out=ot, in0=dn[t2], in1=g.x2_tiles[ti])
            nc.sync.dma_start(out=g.out_d[ti * P:(ti + 1) * P, :], in_=ot)


def _build(flags, reps=1):
    has_qb, has_pb, has_db = flags
    nc = bacc.Bacc("TRN2", target_bir_lowering=False, debug=False, num_devices=8)

    g = _Ctx()
    g.nc = nc
    g.has_qb, g.has_pb, g.has_db = flags
    g.xkv_d = nc.dram_tensor("xkv", [T, E], f32, kind="ExternalInput")
    g.wq_d = nc.dram_tensor("wq", [NE, NPAIR, P, P], bf16, kind="ExternalInput")
    g.wk_d = nc.dram_tensor("wk", [NE, NPAIR, P, P], bf16, kind="ExternalInput")
    g.wv_d = nc.dram_tensor("wv", [NE, P, E], bf16, kind="ExternalInput")
    g.vrow_d = nc.dram_tensor("vrow", [1, H * VW], bf16, kind="ExternalInput")
    g.pw_d = nc.dram_tensor("pw", [NE, P, E], bf16, kind="ExternalInput")
    g.uw_d = nc.dram_tensor("uw", [NE, P, F], bf16, kind="ExternalInput")
    g.ub_d = nc.dram_tensor("ub", [P, NF], f32, kind="ExternalInput")
    g.dw_d = nc.dram_tensor("dw", [NF, P, E], bf16, kind="ExternalInput")
    if has_qb:
        g.qb_d = nc.dram_tensor("qb", [P, NPAIR], f32, kind="ExternalInput")
        g.kb_d = nc.dram_tensor("kb", [P, NPAIR], f32, kind="ExternalInput")
        g.vbrow_d = nc.dram_tensor("vbrow", [1, E], bf16, kind="ExternalInput")
    if has_pb:
        g.pbrow_d = nc.dram_tensor("pbrow", [1, E], f32, kind="ExternalInput")
    if has_db:
        g.dbrow_d = nc.dram_tensor("dbrow", [1, E], f32, kind="ExternalInput")
    g.out_d = nc.dram_tensor("out", [TQ, E], f32, kind="ExternalOutput")

    with tile.TileContext(nc) as tc:
        with (
            tc.tile_pool(name="consts", bufs=1) as consts,
            tc.tile_pool(name="stat", bufs=4) as stat,
            tc.tile_pool(name="catp", bufs=1) as catp,
            tc.tile_pool(name="x2p", bufs=1) as x2p,
            tc.tile_pool(name="h2Tp", bufs=1) as h2Tp,
        ):
            g.consts, g.stat = consts, stat
            _emit_consts(g)
            for _rep in range(reps):
                _emit_all(g, tc, catp, x2p, h2Tp)

    nc.finalize()
    return nc


def _emit_all(g, tc, catp, x2p, h2Tp):
    g.catT = [catp.tile([P, TQ], bf16, name=f"catT{p}") for p in range(NPAIR)]
    g.x2_tiles = [x2p.tile([P, E], f32, name=f"x2_{i}") for i in range(NTS)]
    g.h2T = [h2Tp.tile([P, TQ], bf16, name=f"h2T{c}") for c in range(NE)]

    with (
        tc.tile_pool(name="hp", bufs=4) as hp,
        tc.tile_pool(name="hTp", bufs=1) as hTp,
        tc.tile_pool(name="vaug", bufs=1) as vap,
    ):
        g.hp = hp
        g.hT = [hTp.tile([P, T], bf16, name=f"hT{c}") for c in range(NE)]
        with (
            tc.tile_pool(name="xk", bufs=3) as xkp,
            tc.tile_pool(name="tps", bufs=2, space="PSUM") as tps,
        ):
            _emit_ln1_transpose(g, xkp, tps)

        g.va = [vap.tile([P, H * VW], bf16, name=f"va{s}")
                for s in range(NST)]
        with (
            tc.tile_pool(name="wvp", bufs=1) as wvp,
            tc.tile_pool(name="vps", bufs=2, space="PSUM") as vps,
        ):
            _emit_v(g, wvp, vps)

        with (
            tc.tile_pool(name="wqk", bufs=6) as wqkp,
            tc.tile_pool(name="qtp", bufs=2) as qtp,
            tc.tile_pool(name="ktp", bufs=2) as ktp,
            tc.tile_pool(name="ptp", bufs=4) as ptp,
            tc.tile_pool(name="smp", bufs=2) as smp,
            tc.tile_pool(name="drp", bufs=2, space="DRAM") as drp,
            tc.tile_pool(name="qkps", bufs=1, space="PSUM") as qkps,
            tc.tile_pool(name="scps", bufs=2, space="PSUM") as scps,
            tc.tile_pool(name="atps", bufs=1, space="PSUM") as atps,
        ):
            for p in range(NPAIR):
                qt = qtp.tile([P, TQ], bf16, name="qt")
                kt = ktp.tile([P, T], bf16, name="kt")
                _emit_qkt_pair(g, p, qt, kt, wqkp, qkps)
                _emit_attn_pair(g, p, qt, kt, ptp, smp, drp, scps, atps)

    with (
        tc.tile_pool(name="pwp", bufs=1) as pwp,
        tc.tile_pool(name="uwp", bufs=1) as uwp,
        tc.tile_pool(name="xq2", bufs=3) as xq2p,
        tc.tile_pool(name="h2p", bufs=3) as h2p,
    ):
        with (
            tc.tile_pool(name="pps", bufs=2, space="PSUM") as pps,
            tc.tile_pool(name="t2ps", bufs=2, space="PSUM") as t2ps,
        ):
            _emit_proj_ln2(g, pwp, uwp, xq2p, h2p, pps, t2ps)

        with (
            tc.tile_pool(name="hidp", bufs=6) as hidp,
            tc.tile_pool(name="dwpp", bufs=4) as dwpp,
            tc.tile_pool(name="outp", bufs=3) as outp,
            tc.tile_pool(name="upps", bufs=3, space="PSUM") as upps,
            tc.tile_pool(name="dnps", bufs=1, space="PSUM") as dnps,
        ):
            _emit_mlp(g, hidp, dwpp, outp, upps, dnps)


def _get_nc(flags, reps=1):
    key = (flags, reps)
    if key not in _BUILD_CACHE:
        _BUILD_CACHE[key] = _build(flags, reps)
    return _BUILD_CACHE[key]


def _prep(x, Wq, Wk, Wv, proj_w, proj_b, ln1_g, ln1_b, ln2_g, ln2_b,
          up_w, up_b, down_w, down_b):
    """Host-side shard + weight fold/cast/layout. Returns (flags, in_maps)."""
    bfl = ml_dtypes.bfloat16
    x = np.ascontiguousarray(np.asarray(x, dtype=np.float32))
    Wq = np.asarray(Wq, np.float32)
    Wk = np.asarray(Wk, np.float32)
    Wv = np.asarray(Wv, np.float32)
    g1 = np.asarray(ln1_g, np.float32)
    b1 = np.asarray(ln1_b, np.float32)
    g2 = np.asarray(ln2_g, np.float32)
    b2 = np.asarray(ln2_b, np.float32)
    proj_w = np.asarray(proj_w, np.float32)
    up_w = np.asarray(up_w, np.float32)
    down_w = np.asarray(down_w, np.float32)

    # [H, E, D] -> [E, H*D]; fold attention scale into Q, LN1 gain into all
    wq_all = (Wq * (D ** -0.5)).transpose(1, 0, 2).reshape(E, E)
    wk_all = Wk.transpose(1, 0, 2).reshape(E, E)
    wv_all = Wv.transpose(1, 0, 2).reshape(E, E)
    qb_vec = b1 @ wq_all
    kb_vec = b1 @ wk_all
    vb_vec = b1 @ wv_all
    wq_f = g1[:, None] * wq_all
    wk_f = g1[:, None] * wk_all
    wv_f = g1[:, None] * wv_all

    def _pair_chunks(w):  # [E, E] -> [NE, NPAIR, P, P]
        return np.ascontiguousarray(
            w.reshape(NE, P, NPAIR, P).transpose(0, 2, 1, 3).astype(bfl)
        )

    vrow = np.zeros((1, H * VW), np.float32)
    vrow.reshape(H, VW)[:, D] = 1.0

    uw_f = g2[:, None] * up_w
    ub_f = np.asarray(up_b, np.float32) + b2 @ up_w

    has_qb = bool(np.any(b1 != 0))
    has_pb = bool(np.any(np.asarray(proj_b) != 0))
    has_db = bool(np.any(np.asarray(down_b) != 0))
    flags = (has_qb, has_pb, has_db)

    shared = {
        "wq": _pair_chunks(wq_f),
        "wk": _pair_chunks(wk_f),
        "wv": np.ascontiguousarray(wv_f.reshape(NE, P, E).astype(bfl)),
        "vrow": vrow.astype(bfl),
        "pw": np.ascontiguousarray(proj_w.reshape(NE, P, E).astype(bfl)),
        "uw": np.ascontiguousarray(uw_f.reshape(NE, P, F).astype(bfl)),
        "ub": np.ascontiguousarray(ub_f.reshape(NF, P).T.astype(np.float32)),
        "dw": np.ascontiguousarray(down_w.reshape(NF, P, E).astype(bfl)),
    }
    if has_qb:
        shared["qb"] = np.ascontiguousarray(
            qb_vec.reshape(NPAIR, P).T.astype(np.float32))
        shared["kb"] = np.ascontiguousarray(
            kb_vec.reshape(NPAIR, P).T.astype(np.float32))
        shared["vbrow"] = vb_vec.reshape(1, E).astype(bfl)
    if has_pb:
        shared["pbrow"] = np.asarray(proj_b, np.float32).reshape(1, E)
    if has_db:
        shared["dbrow"] = np.asarray(down_b, np.float32).reshape(1, E)

    in_maps = []
    for c in range(8):
        b, half = c // 2, c % 2
        xb = x[b]
        if half == 1:
            xb = np.concatenate([xb[TQ:], xb[:TQ]], axis=0)
        in_maps.append({"xkv": np.ascontiguousarray(xb), **shared})
    return flags, in_maps


def kernel(**inputs) -> np.ndarray:
    flags, in_maps = _prep(**inputs)
    nc = _get_nc(flags)
    res = run_bass_kernel_spmd(nc, in_maps, core_ids=list(range(8)))
    out = np.empty((B, T, E), np.float32)
    for c in range(8):
        b, half = c // 2, c % 2
        out[b, half * TQ:(half + 1) * TQ, :] = res.results[c]["out"]
    return out


# revision 14
# speedup vs baseline: 161.3795x; 2.0963x over previous
"""Trainium2 Bass kernel for a dense transformer block (nn_Block_120259084502).

Contract: kernel(**inputs) takes the FULL unsharded inputs (numpy, fp32) and
returns the FULL output [4, 2048, 1024] fp32. Internally shards across 8
NeuronCores: core c handles batch c//2, query-token half c%2. Each core
receives its batch's full 2048 tokens (rolled so its own 1024 query tokens
come first) and computes K/V for all of them locally, so no collectives are
needed (attention context = full batch; softmax is order-invariant so the
roll is harmless).

All heavy math runs on-device in bf16 matmuls with fp32 PSUM accumulation;
LayerNorm statistics and both residual adds stay fp32. LN gains and the
attention 1/sqrt(D) scale are folded into the weights on the host.
"""

import numpy as np
import ml_dtypes

import concourse.bacc as bacc
import concourse.tile as tile
from concourse import mybir
from concourse.bass_utils import run_bass_kernel_spmd
from concourse.masks import make_identity

bf16 = mybir.dt.bfloat16
f32 = mybir.dt.float32
AF = mybir.ActivationFunctionType
ALU = mybir.AluOpType

P = 128
B, T, E, H, D = 4, 2048, 1024, 16, 64
F = 4 * E                    # 4096 MLP hidden
TQ = T // 2                  # 1024 own query tokens per core
NE = E // P                  # 8 e-chunks
NPAIR = H // 2               # 8 head pairs
NST = T // P                 # 16 context-token tiles
NTS = TQ // P                # 8 own-token tiles
NF = F // P                  # 32 f-chunks
VW = D + 1                   # per-head V width incl. ones column
LN_EPS = 1e-5

_BUILD_CACHE = {}


class _Ctx:
    """Shared build state passed between phase emitters."""
    pass


def _emit_ln(g, xt, out_bf):
    nc = g.nc
    st = g.stat.tile([P, 2, nc.vector.BN_STATS_DIM], f32, name="bnst")
    xv = xt.rearrange("p (s g) -> p s g", s=2)
    nc.vector.bn_stats(out=st[:, 0, :], in_=xv[:, 0, :])
    nc.vector.bn_stats(out=st[:, 1, :], in_=xv[:, 1, :])
    mv = g.stat.tile([P, nc.vector.BN_AGGR_DIM], f32, name="bnmv")
    nc.vector.bn_aggr(out=mv, in_=st)
    rstd = g.stat.tile([P, 1], f32, name="bnrs")
    nc.scalar.activation(out=rstd, in_=mv[:, 1:2], func=AF.Sqrt, bias=g.eps_t)
    nc.vector.reciprocal(out=rstd, in_=rstd)
    nc.vector.tensor_scalar(
        out=out_bf, in0=xt, scalar1=mv[:, 0:1], scalar2=rstd,
        op0=ALU.subtract, op1=ALU.mult,
    )


def _emit_consts(g):
    nc, consts = g.nc, g.consts
    g.ident = consts.tile([P, P], bf16, name="ident")
    make_identity(nc, g.ident)
    g.eps_t = consts.tile([P, 1], f32, name="eps")
    nc.vector.memset(g.eps_t, LN_EPS)
    g.ub_sb = consts.tile([P, NF], f32, name="ubsb")
    nc.sync.dma_start(out=g.ub_sb, in_=g.ub_d[:, :])
    if g.has_qb:
        g.qb_sb = consts.tile([P, NPAIR], f32, name="qbsb")
        nc.sync.dma_start(out=g.qb_sb, in_=g.qb_d[:, :])
        g.kb_sb = consts.tile([P, NPAIR], f32, name="kbsb")
        nc.sync.dma_start(out=g.kb_sb, in_=g.kb_d[:, :])
        g.vb_bc = consts.tile([P, E], bf16, name="vbbc")
        nc.gpsimd.dma_start(
            out=g.vb_bc, in_=g.vbrow_d.ap()[0:1, :].partition_broadcast(P)[:, 0, :]
        )
    if g.has_pb:
        g.pb_bc = consts.tile([P, E], f32, name="pbbc")
        nc.gpsimd.dma_start(
            out=g.pb_bc, in_=g.pbrow_d.ap()[0:1, :].partition_broadcast(P)[:, 0, :]
        )
    if g.has_db:
        g.db_bc = consts.tile([P, E], f32, name="dbbc")
        nc.gpsimd.dma_start(
            out=g.db_bc, in_=g.dbrow_d.ap()[0:1, :].partition_broadcast(P)[:, 0, :]
        )


def _emit_ln1_transpose(g, xkp, tps):
    """Load x, LN1, PE-transpose h into e-major hT."""
    nc = g.nc
    for i in range(NST):
        xt = xkp.tile([P, E], f32, name="xk")
        nc.sync.dma_start(out=xt, in_=g.xkv_d[i * P:(i + 1) * P, :])
        ht = g.hp.tile([P, E], bf16, name="h")
        _emit_ln(g, xt, ht)
        for c in range(NE):
            tp = tps.tile([P, P], bf16, name="tp")
            nc.tensor.transpose(tp, ht[:, c * P:(c + 1) * P], g.ident)
            nc.vector.tensor_copy(out=g.hT[c][:, i * P:(i + 1) * P], in_=tp)


def _emit_v(g, wvp, vps):
    """V in natural [s, d] layout for all heads, with ones column per head."""
    nc = g.nc
    wv_sb = []
    for c in range(NE):
        w = wvp.tile([P, E], bf16, name=f"wv{c}")
        nc.sync.dma_start(out=w, in_=g.wv_d[c])
        wv_sb.append(w)
    for s in range(NST):
        nc.gpsimd.dma_start(
            out=g.va[s],
            in_=g.vrow_d.ap()[0:1, :].partition_broadcast(P)[:, 0, :],
        )
        pv = [vps.tile([P, 512], f32, name=f"pv{j}") for j in range(2)]
        for c in range(NE):
            for j in range(2):
                nc.tensor.matmul(
                    pv[j], g.hT[c][:, s * P:(s + 1) * P],
                    wv_sb[c][:, j * 512:(j + 1) * 512],
                    start=(c == 0), stop=(c == NE - 1),
                )
        for j in range(2):
            dst = g.va[s].rearrange("p (h c) -> p h c", c=VW)[
                :, j * 8:(j + 1) * 8, 0:D
            ]
            src = pv[j].rearrange("p (h d) -> p h d", d=D)
            if g.has_qb:
                vb_view = g.vb_bc.rearrange("p (h d) -> p h d", d=D)[
                    :, j * 8:(j + 1) * 8, :
                ]
                nc.vector.tensor_add(out=dst, in0=src, in1=vb_view)
            else:
                nc.vector.tensor_copy(out=dst, in_=src)


def _emit_qkt_pair(g, p, qt, kt, wqkp, qkps):
    """Q^T and K^T for head pair p: [128 (2 heads x 64d), tokens]."""
    nc = g.nc
    psq = [qkps.tile([P, 512], f32, name=f"ps{j}") for j in range(2)]
    for c in range(NE):
        wsl = wqkp.tile([P, P], bf16, name="wsl")
        nc.sync.dma_start(out=wsl, in_=g.wq_d[c, p])
        for j in range(2):
            nc.tensor.matmul(
                psq[j], wsl, g.hT[c][:, j * 512:(j + 1) * 512],
                start=(c == 0), stop=(c == NE - 1),
            )
    for j in range(2):
        dst = qt[:, j * 512:(j + 1) * 512]
        if g.has_qb:
            nc.vector.tensor_scalar(
                out=dst, in0=psq[j], scalar1=g.qb_sb[:, p:p + 1], op0=ALU.add
            )
        else:
            nc.vector.tensor_copy(out=dst, in_=psq[j])
    for sh in range(2):
        psk = [qkps.tile([P, 512], f32, name=f"ps{j}") for j in range(2)]
        for c in range(NE):
            wsl = wqkp.tile([P, P], bf16, name="wsl")
            nc.sync.dma_start(out=wsl, in_=g.wk_d[c, p])
            for j in range(2):
                s0 = (sh * 2 + j) * 512
                nc.tensor.matmul(
                    psk[j], wsl, g.hT[c][:, s0:s0 + 512],
                    start=(c == 0), stop=(c == NE - 1),
                )
        for j in range(2):
            s0 = (sh * 2 + j) * 512
            dst = kt[:, s0:s0 + 512]
            if g.has_qb:
                nc.vector.tensor_scalar(
                    out=dst, in0=psk[j], scalar1=g.kb_sb[:, p:p + 1], op0=ALU.add
                )
            else:
                nc.vector.tensor_copy(out=dst, in_=psk[j])


def _emit_attn_pair(g, p, qt, kt, ptp, smp, scps, atps):
    """Scores (transposed), exp, attn^T + softmax denom, normalize -> catT."""
    nc = g.nc  # noqa
    for th in range(2):
        tcols = slice(th * 512, (th + 1) * 512)
        at0 = atps.tile([D + 1, 512], f32, name="ps0")
        at1 = atps.tile([D + 1, 512], f32, name="ps1")
        for s in range(NST):
            scols = slice(s * P, (s + 1) * P)
            sc0 = scps.tile([P, 512], f32, name="sc0")
            sc1 = scps.tile([P, 512], f32, name="sc1")
            # S^T[s,t] = (K^T slice).T @ Q^T slice; the two heads live on
            # row-groups 0-63 / 64-127 so the matmuls pack concurrently.
            nc.tensor.matmul(sc0, kt[0:D, scols], qt[0:D, tcols],
                             start=True, stop=True)
            nc.tensor.matmul(sc1, kt[D:2 * D, scols], qt[D:2 * D, tcols],
                             start=True, stop=True)
            pt0 = ptp.tile([P, 512], bf16, name="pt0")
            pt1 = ptp.tile([P, 512], bf16, name="pt1")
            nc.scalar.activation(out=pt0, in_=sc0, func=AF.Exp)
            nc.scalar.activation(out=pt1, in_=sc1, func=AF.Exp)
            nc.tensor.matmul(
                at0, g.va[s][:, (2 * p) * VW:(2 * p) * VW + VW], pt0,
                start=(s == 0), stop=(s == NST - 1),
            )
            nc.tensor.matmul(
                at1, g.va[s][:, (2 * p + 1) * VW:(2 * p + 1) * VW + VW], pt1,
                start=(s == 0), stop=(s == NST - 1),
            )
        se0 = smp.tile([1, 512], f32, name="se0")
        se1 = smp.tile([1, 512], f32, name="se1")
        nc.vector.reciprocal(out=se0, in_=at0[D:D + 1, :])
        nc.vector.reciprocal(out=se1, in_=at1[D:D + 1, :])
        rb0 = smp.tile([D, 512], f32, name="rb0")
        rb1 = smp.tile([D, 512], f32, name="rb1")
        nc.gpsimd.partition_broadcast(rb0, se0)
        nc.gpsimd.partition_broadcast(rb1, se1)
        nc.vector.tensor_mul(out=g.catT[p][0:D, tcols], in0=at0[0:D, :], in1=rb0)
        nc.vector.tensor_mul(out=g.catT[p][D:2 * D, tcols], in0=at1[0:D, :], in1=rb1)


def _emit_proj_ln2(g, uwp, xq2p, h2p, pps, t2ps):
    nc = g.nc
    pw_sb = g.pw_sb
    g.uw_sb = []
    for c in range(NE):  # prefetch MLP up-weights while proj runs
        w = uwp.tile([P, F], bf16, name=f"uw{c}")
        nc.sync.dma_start(out=w, in_=g.uw_d[c])
        g.uw_sb.append(w)
    for ts in range(NTS):
        trows = slice(ts * P, (ts + 1) * P)
        xres = xq2p.tile([P, E], f32, name="xres")
        nc.sync.dma_start(out=xres, in_=g.xkv_d[ts * P:(ts + 1) * P, :])
        psy = [pps.tile([P, 512], f32, name=f"py{j}") for j in range(2)]
        for c in range(NE):
            for j in range(2):
                nc.tensor.matmul(
                    psy[j], g.catT[c][:, trows],
                    pw_sb[c][:, j * 512:(j + 1) * 512],
                    start=(c == 0), stop=(c == NE - 1),
                )
        x2 = g.x2_tiles[ts]
        for j in range(2):
            jc = slice(j * 512, (j + 1) * 512)
            if g.has_pb:
                nc.vector.tensor_add(out=x2[:, jc], in0=psy[j], in1=g.pb_bc[:, jc])
                nc.vector.tensor_add(out=x2[:, jc], in0=x2[:, jc],
                                     in1=xres[:, jc])
            else:
                nc.vector.tensor_add(out=x2[:, jc], in0=psy[j],
                                     in1=xres[:, jc])
        h2 = h2p.tile([P, E], bf16, name="h2")
        _emit_ln(g, x2, h2)
        for c in range(NE):
            tp = t2ps.tile([P, P], bf16, name="t2")
            nc.tensor.transpose(tp, h2[:, c * P:(c + 1) * P], g.ident)
            nc.vector.tensor_copy(out=g.h2T[c][:, trows], in_=tp)


def _emit_mlp(g, hidp, dwpp, outp, upps, dnps):
    nc = g.nc
    TQQ = 256  # token quarter
    for q in range(4):
        qcols = slice(q * TQQ, (q + 1) * TQQ)
        dn = [dnps.tile([P, E], f32, name=f"dn{j}") for j in range(2)]
        for f in range(NF):
            pu = upps.tile([P, TQQ], f32, name="pu")
            for c in range(NE):
                nc.tensor.matmul(
                    pu, g.uw_sb[c][:, f * P:(f + 1) * P], g.h2T[c][:, qcols],
                    start=(c == 0), stop=(c == NE - 1),
                )
            hid = hidp.tile([P, TQQ], bf16, name="hid")
            nc.scalar.activation(out=hid, in_=pu, func=AF.Relu,
                                 bias=g.ub_sb[:, f:f + 1])
            dwt = dwpp.tile([P, E], bf16, name="dwt")
            nc.sync.dma_start(out=dwt, in_=g.dw_d[f])
            for t2 in range(2):
                for j in range(2):
                    nc.tensor.matmul(
                        dn[t2][:, j * 512:(j + 1) * 512],
                        hid[:, t2 * P:(t2 + 1) * P],
                        dwt[:, j * 512:(j + 1) * 512],
                        start=(f == 0), stop=(f == NF - 1),
                    )
        for t2 in range(2):
            ti = q * 2 + t2
            ot = outp.tile([P, E], f32, name="ot")
            if g.has_db:
                nc.vector.tensor_add(out=ot, in0=dn[t2], in1=g.db_bc)
                nc.vector.tensor_add(out=ot, in0=ot, in1=g.x2_tiles[ti])
            else:
                nc.vector.tensor_add(out=ot, in0=dn[t2], in1=g.x2_tiles[ti])
            nc.sync.dma_start(out=g.out_d[ti * P:(ti + 1) * P, :], in_=ot)


def _build(flags, reps=1):
    has_qb, has_pb, has_db = flags
    nc = bacc.Bacc("TRN2", target_bir_lowering=False, debug=False, num_devices=8)

    g = _Ctx()
    g.nc = nc
    g.has_qb, g.has_pb, g.has_db = flags
    g.xkv_d = nc.dram_tensor("xkv", [T, E], f32, kind="ExternalInput")
    g.wq_d = nc.dram_tensor("wq", [NE, NPAIR, P, P], bf16, kind="ExternalInput")
    g.wk_d = nc.dram_tensor("wk", [NE, NPAIR, P, P], bf16, kind="ExternalInput")
    g.wv_d = nc.dram_tensor("wv", [NE, P, E], bf16, kind="ExternalInput")
    g.vrow_d = nc.dram_tensor("vrow", [1, H * VW], bf16, kind="ExternalInput")
    g.pw_d = nc.dram_tensor("pw", [NE, P, E], bf16, kind="ExternalInput")
    g.uw_d = nc.dram_tensor("uw", [NE, P, F], bf16, kind="ExternalInput")
    g.ub_d = nc.dram_tensor("ub", [P, NF], f32, kind="ExternalInput")
    g.dw_d = nc.dram_tensor("dw", [NF, P, E], bf16, kind="ExternalInput")
    if has_qb:
        g.qb_d = nc.dram_tensor("qb", [P, NPAIR], f32, kind="ExternalInput")
        g.kb_d = nc.dram_tensor("kb", [P, NPAIR], f32, kind="ExternalInput")
        g.vbrow_d = nc.dram_tensor("vbrow", [1, E], bf16, kind="ExternalInput")
    if has_pb:
        g.pbrow_d = nc.dram_tensor("pbrow", [1, E], f32, kind="ExternalInput")
    if has_db:
        g.dbrow_d = nc.dram_tensor("dbrow", [1, E], f32, kind="ExternalInput")
    g.out_d = nc.dram_tensor("out", [TQ, E], f32, kind="ExternalOutput")

    with tile.TileContext(nc) as tc:
        with (
            tc.tile_pool(name="consts", bufs=1) as consts,
            tc.tile_pool(name="stat", bufs=4) as stat,
            tc.tile_pool(name="catp", bufs=1) as catp,
            tc.tile_pool(name="x2p", bufs=1) as x2p,
            tc.tile_pool(name="h2Tp", bufs=1) as h2Tp,
        ):
            g.consts, g.stat = consts, stat
            _emit_consts(g)
            for _rep in range(reps):
                _emit_all(g, tc, catp, x2p, h2Tp)

    nc.finalize()
    return nc


def _emit_all(g, tc, catp, x2p, h2Tp):
    g.catT = [catp.tile([P, TQ], bf16, name=f"catT{p}") for p in range(NPAIR)]
    g.x2_tiles = [x2p.tile([P, E], f32, name=f"x2_{i}") for i in range(NTS)]
    g.h2T = [h2Tp.tile([P, TQ], bf16, name=f"h2T{c}") for c in range(NE)]

    g.pwp = tc.alloc_tile_pool(name="pwp", bufs=1)
    with (
        tc.tile_pool(name="hp", bufs=4) as hp,
        tc.tile_pool(name="hTp", bufs=1) as hTp,
        tc.tile_pool(name="vaug", bufs=1) as vap,
    ):
        g.hp = hp
        g.hT = [hTp.tile([P, T], bf16, name=f"hT{c}") for c in range(NE)]
        with (
            tc.tile_pool(name="xk", bufs=3) as xkp,
            tc.tile_pool(name="tps", bufs=2, space="PSUM") as tps,
        ):
            _emit_ln1_transpose(g, xkp, tps)

        g.va = [vap.tile([P, H * VW], bf16, name=f"va{s}")
                for s in range(NST)]
        with (
            tc.tile_pool(name="wvp", bufs=1) as wvp,
            tc.tile_pool(name="vps", bufs=4, space="PSUM") as vps,
        ):
            _emit_v(g, wvp, vps)

        with (
            tc.tile_pool(name="wqk", bufs=6) as wqkp,
            tc.tile_pool(name="qtp", bufs=2) as qtp,
            tc.tile_pool(name="ktp", bufs=2) as ktp,
            tc.tile_pool(name="ptp", bufs=4) as ptp,
            tc.tile_pool(name="smp", bufs=2) as smp,
            tc.tile_pool(name="qaps", bufs=2, space="PSUM") as qaps,
            tc.tile_pool(name="scps", bufs=2, space="PSUM") as scps,
        ):
            for p in range(NPAIR):
                qt = qtp.tile([P, TQ], bf16, name="qt")
                kt = ktp.tile([P, T], bf16, name="kt")
                _emit_qkt_pair(g, p, qt, kt, wqkp, qaps)
                _emit_attn_pair(g, p, qt, kt, ptp, smp, scps, qaps)
                if p == 0:
                    # prefetch proj weights on the idle SWDGE queue so the
                    # proj phase doesn't stall on them later
                    g.pw_sb = []
                    for c in range(NE):
                        w = g.pwp.tile([P, E], bf16, name=f"pw{c}")
                        g.nc.gpsimd.dma_start(out=w, in_=g.pw_d[c])
                        g.pw_sb.append(w)

    with (
        tc.tile_pool(name="uwp", bufs=1) as uwp,
        tc.tile_pool(name="xq2", bufs=3) as xq2p,
        tc.tile_pool(name="h2p", bufs=3) as h2p,
    ):
        with (
            tc.tile_pool(name="pps", bufs=2, space="PSUM") as pps,
            tc.tile_pool(name="t2ps", bufs=2, space="PSUM") as t2ps,
        ):
            _emit_proj_ln2(g, uwp, xq2p, h2p, pps, t2ps)

        with (
            tc.tile_pool(name="hidp", bufs=6) as hidp,
            tc.tile_pool(name="dwpp", bufs=4) as dwpp,
            tc.tile_pool(name="outp", bufs=3) as outp,
            tc.tile_pool(name="upps", bufs=3, space="PSUM") as upps,
            tc.tile_pool(name="dnps", bufs=1, space="PSUM") as dnps,
        ):
            _emit_mlp(g, hidp, dwpp, outp, upps, dnps)
    g.pwp.release()


def _get_nc(flags, reps=1):
    key = (flags, reps)
    if key not in _BUILD_CACHE:
        _BUILD_CACHE[key] = _build(flags, reps)
    return _BUILD_CACHE[key]


def _prep(x, Wq, Wk, Wv, proj_w, proj_b, ln1_g, ln1_b, ln2_g, ln2_b,
          up_w, up_b, down_w, down_b):
    """Host-side shard + weight fold/cast/layout. Returns (flags, in_maps)."""
    bfl = ml_dtypes.bfloat16
    x = np.ascontiguousarray(np.asarray(x, dtype=np.float32))
    Wq = np.asarray(Wq, np.float32)
    Wk = np.asarray(Wk, np.float32)
    Wv = np.asarray(Wv, np.float32)
    g1 = np.asarray(ln1_g, np.float32)
    b1 = np.asarray(ln1_b, np.float32)
    g2 = np.asarray(ln2_g, np.float32)
    b2 = np.asarray(ln2_b, np.float32)
    proj_w = np.asarray(proj_w, np.float32)
    up_w = np.asarray(up_w, np.float32)
    down_w = np.asarray(down_w, np.float32)

    # [H, E, D] -> [E, H*D]; fold attention scale into Q, LN1 gain into all
    wq_all = (Wq * (D ** -0.5)).transpose(1, 0, 2).reshape(E, E)
    wk_all = Wk.transpose(1, 0, 2).reshape(E, E)
    wv_all = Wv.transpose(1, 0, 2).reshape(E, E)
    qb_vec = b1 @ wq_all
    kb_vec = b1 @ wk_all
    vb_vec = b1 @ wv_all
    wq_f = g1[:, None] * wq_all
    wk_f = g1[:, None] * wk_all
    wv_f = g1[:, None] * wv_all

    def _pair_chunks(w):  # [E, E] -> [NE, NPAIR, P, P]
        return np.ascontiguousarray(
            w.reshape(NE, P, NPAIR, P).transpose(0, 2, 1, 3).astype(bfl)
        )

    vrow = np.zeros((1, H * VW), np.float32)
    vrow.reshape(H, VW)[:, D] = 1.0

    uw_f = g2[:, None] * up_w
    ub_f = np.asarray(up_b, np.float32) + b2 @ up_w

    has_qb = bool(np.any(b1 != 0))
    has_pb = bool(np.any(np.asarray(proj_b) != 0))
    has_db = bool(np.any(np.asarray(down_b) != 0))
    flags = (has_qb, has_pb, has_db)

    shared = {
        "wq": _pair_chunks(wq_f),
        "wk": _pair_chunks(wk_f),
        "wv": np.ascontiguousarray(wv_f.reshape(NE, P, E).astype(bfl)),
        "vrow": vrow.astype(bfl),
        "pw": np.ascontiguousarray(proj_w.reshape(NE, P, E).astype(bfl)),
        "uw": np.ascontiguousarray(uw_f.reshape(NE, P, F).astype(bfl)),
        "ub": np.ascontiguousarray(ub_f.reshape(NF, P).T.astype(np.float32)),
        "dw": np.ascontiguousarray(down_w.reshape(NF, P, E).astype(bfl)),
    }
    if has_qb:
        shared["qb"] = np.ascontiguousarray(
            qb_vec.reshape(NPAIR, P).T.astype(np.float32))
        shared["kb"] = np.ascontiguousarray(
            kb_vec.reshape(NPAIR, P).T.astype(np.float32))
        shared["vbrow"] = vb_vec.reshape(1, E).astype(bfl)
    if has_pb:
        shared["pbrow"] = np.asarray(proj_b, np.float32).reshape(1, E)
    if has_db:
        shared["dbrow"] = np.asarray(down_b, np.float32).reshape(1, E)

    in_maps = []
    for c in range(8):
        b, half = c // 2, c % 2
        xb = x[b]
        if half == 1:
            xb = np.concatenate([xb[TQ:], xb[:TQ]], axis=0)
        in_maps.append({"xkv": np.ascontiguousarray(xb), **shared})
    return flags, in_maps


def kernel(**inputs) -> np.ndarray:
    flags, in_maps = _prep(**inputs)
    nc = _get_nc(flags)
    res = run_bass_kernel_spmd(nc, in_maps, core_ids=list(range(8)))
    out = np.empty((B, T, E), np.float32)
    for c in range(8):
        b, half = c // 2, c % 2
        out[b, half * TQ:(half + 1) * TQ, :] = res.results[c]["out"]
    return out
